# revision 7
# baseline (speedup 1.0000x reference)
"""Trainium2 Bass kernel for nn_EquiLinearLayer (fp8 DoubleRow, v3).

Computes  out[s,n,j,y] = sum_{i,x,b} weights[j,i,b] * blade[b,x,y] * x[s,n,i,x]
for x:[8,2048,512,16] f32, weights:[512,512,9] f32, blade:[9,16,16] f32.

v3 strategy (fp8e4m3 + MatmulPerfMode.DoubleRow = 2 moving rows/PE-cycle):
  * Host computes xb[pt,i,b,y] = sum_x x[pt,i,x]*blade[b,x,y] (19.3 GMACs,
    one [8.4M,16]@[16,144] sgemm), exactly as the fp16 baseline.
  * Accuracy at 3 mantissa bits: quantizing W in fp8 directly gives
    ~2.7e-2 max rel err (gate: 2e-2). Two exact tricks claw it back:
      - mean-shift: W' = W - m_j with m_j = mean_{i,b} W[j,i,b]. Both
        quantization error terms scale with E[W'^2] = 1/4 * E[W^2]
        (W~U[0,1]) => error std halves. The exact rank-1 correction
        m_j * S[pt,y], S = sum_{i,b} xb, is added on the DVE engine:
        one scalar_tensor_tensor: out = (m64b * S_col) + psum.
      - scales: W'*16, xb*4 (pure exponent shifts); psum = 64*out fits
        fp16; host divides by 64.
    Measured on the real seed-0 inputs: rel_err 1.679e-2, HW and host sim
    agree to 4 digits (see fp8_sim.py). fp16 baseline was 2.9e-4 @ 2.46ms;
    this runs ~1.10ms (PE floor 0.983ms + ~57 cyc/matmul LdWeights bubble).
  * Device per group of 8 points: 18 DoubleRow matmuls, each contracting
    TWO 128-chunks of (i,b): lhsT = xb pair [128,2,128], rhs = W pair
    [128,2,512], psum [128(dpt,y), 512 j] accumulates all 36 chunks.
    PE cost 18*512 = 9216 cyc/group vs fp16's 36*512 = 18432.
  * Output fp16 [128,512] per group (halves out-DMA); host /64 + permute.
"""

from contextlib import ExitStack

import numpy as np
import ml_dtypes

import concourse.bass as bass
import concourse.mybir as mybir
import concourse.tile as tile
from concourse import bacc
from concourse.bass_utils import run_bass_kernel_spmd

BATCH, NPTS, C, MV, BL = 8, 2048, 512, 16, 9
J = 512
N_CORES = 8
TOTAL_PTS = BATCH * NPTS             # 16384
GROUPS = TOTAL_PTS // 8              # 2048 groups of 8 points
GROUPS_PER_CORE = GROUPS // N_CORES  # 256
NCHUNK = 36                          # (i,b) contraction chunks of 128
NPAIR = NCHUNK // 2

F32 = mybir.dt.float32
FP16 = mybir.dt.float16
FP8 = mybir.dt.float8e4
FP8_NP = ml_dtypes.float8_e4m3

S_W = 16.0     # weight scale (W' in [-0.5,0.5] -> [-8,8])
S_X = 4.0      # xb scale (|xb| < ~12 -> < 48)
S_OUT = S_W * S_X

DR = mybir.MatmulPerfMode.DoubleRow


def build_program(groups: int = GROUPS_PER_CORE, repeats: int = 1,
                  psum_bufs: int = 4, xb_bufs: int = 4,
                  sw_interleave: bool = False) -> bass.Bass:
    nc = bacc.Bacc(trn_type="TRN2", target_bir_lowering=False, debug=False)
    perf_mode = (mybir.MatmulPerfMode.DoubleRowSwInterleave if sw_interleave
                 else DR)

    xb_d = nc.dram_tensor("XB", [groups, 128, NCHUNK * 128], FP8,
                          kind="ExternalInput")
    w_d = nc.dram_tensor("W2", [NPAIR, 128, 2, J], FP8, kind="ExternalInput")
    m_d = nc.dram_tensor("M64B", [128, J], F32, kind="ExternalInput")
    s_d = nc.dram_tensor("SALL", [128, groups], F32, kind="ExternalInput")
    out_d = nc.dram_tensor("outT", [groups, 128, J], FP16, kind="ExternalOutput")

    with tile.TileContext(nc) as tc, ExitStack() as ctx:
        const = ctx.enter_context(tc.tile_pool(name="const", bufs=1))
        xbp = ctx.enter_context(tc.tile_pool(name="xbp", bufs=xb_bufs))
        osb = ctx.enter_context(tc.tile_pool(name="osb", bufs=3))
        ps2 = ctx.enter_context(tc.tile_pool(name="ps2", bufs=psum_bufs, space="PSUM"))

        w2t = []
        for t in range(NPAIR):
            w = const.tile([128, 2, J], FP8, tag=f"w2_{t}")
            nc.sync.dma_start(out=w[:], in_=w_d[t])
            w2t.append(w)
        m64b = const.tile([128, J], F32, tag="m64b")
        nc.sync.dma_start(out=m64b[:], in_=m_d[:])
        s_all = const.tile([128, groups], F32, tag="sall")
        nc.sync.dma_start(out=s_all[:], in_=s_d[:])

        dma_engines = [nc.sync, nc.scalar]
        for n, g in enumerate([g for _ in range(repeats) for g in range(groups)]):
            xt = xbp.tile([128, NCHUNK, 128], FP8, tag="xb")
            dma_engines[n % 2].dma_start(out=xt[:], in_=xb_d[g])
            p2 = ps2.tile([128, J], F32, tag="p2")
            for t in range(NPAIR):
                nc.tensor.matmul(
                    p2[:],
                    xt[:, 2 * t:2 * t + 2, :],
                    w2t[t][:],
                    start=(t == 0), stop=(t == NPAIR - 1),
                    perf_mode=perf_mode,
                )
            ot = osb.tile([128, J], FP16, tag="osb")
            # ot = (m64b * S[:,g]) + psum   (the rank-1 mean correction)
            nc.vector.scalar_tensor_tensor(
                ot[:], m64b[:], s_all[:, g:g + 1], p2[:],
                op0=mybir.AluOpType.mult, op1=mybir.AluOpType.add,
            )
            nc.gpsimd.dma_start(out=out_d[g], in_=ot[:])

    nc.compile()
    return nc


def prep_inputs(x: np.ndarray, weights: np.ndarray, blade: np.ndarray):
    """Host prep.

    Returns xb8 [GROUPS,128,4608] fp8, w8 [18,128,2,512] fp8,
            m64b [128,512] f32, s_all [128,GROUPS] f32 (global; shard cols).
    """
    import torch
    xt = torch.from_numpy(np.ascontiguousarray(x, np.float32))
    # rows ordered (g, il, ic, dpt) so each group tile is 128 contiguous lines
    xr = (xt.reshape(GROUPS, 8, 4, 128, MV).permute(0, 3, 2, 1, 4)
          .reshape(-1, MV).contiguous())
    bm = (torch.from_numpy(np.ascontiguousarray(blade, np.float32))
          .permute(1, 0, 2).reshape(MV, BL * MV).contiguous())
    xb = xr @ bm                                   # [G*128*4*8, 144] f32
    # S[g, dpt, y] = sum_{il, ic, b} xb  (exact, fp32)
    S = (xb.reshape(GROUPS, 128, 4, 8, BL, MV)
         .sum(dim=(1, 2, 4)))                      # [G, 8, 16]
    s_all = S.reshape(GROUPS, 128).T.contiguous().numpy()  # [128, GROUPS]

    xq = (xb * S_X).to(torch.float8_e4m3fn)        # bit-compat w/ ml float8_e4m3
    # relayout lines (g,il): (ic, dpt, b, y) -> (ic, b, dpt, y) via int64 view
    # (y-row = 16 fp8 bytes = 2 int64)
    xp = (xq.view(torch.int8).reshape(GROUPS, 128, 4, 8, BL, MV)
          .view(torch.int64).reshape(GROUPS, 128, 4, 8, BL, 2)
          .permute(0, 1, 2, 4, 3, 5).contiguous()
          .view(torch.int8))
    xb8 = xp.numpy().view(FP8_NP).reshape(GROUPS, 128, NCHUNK * 128)

    wt = torch.from_numpy(np.ascontiguousarray(weights, np.float32))
    m = wt.mean(dim=(1, 2))                        # [j]
    wp = wt - m[:, None, None]
    # Wc[c=(ic*9+b), il, j] = W'[j, ic*128+il, b] * S_W
    wc = (wp.permute(1, 2, 0).reshape(4, 128, BL, J)
          .permute(0, 2, 1, 3).reshape(NCHUNK, 128, J) * S_W)
    w8t = wc.to(torch.float8_e4m3fn)
    w8 = (w8t.view(torch.int8).reshape(NPAIR, 2, 128, J)
          .permute(0, 2, 1, 3).contiguous().numpy().view(FP8_NP))

    m64b = np.tile((m.numpy() * S_OUT)[None, :], (128, 1)).astype(np.float32)
    return xb8, w8, m64b, s_all


def unprep_output(outT_all: np.ndarray) -> np.ndarray:
    """outT_all [GROUPS,128,J] fp16 -> out [BATCH,NPTS,J,MV] f32."""
    import torch
    t = torch.from_numpy(outT_all.view(np.float16)).float() / S_OUT
    return (t.reshape(GROUPS, 8, MV, J).permute(0, 1, 3, 2).contiguous()
            .reshape(BATCH, NPTS, J, MV).numpy())


_NC_CACHE = {}


def _get_program():
    key = (GROUPS_PER_CORE, 1)
    if key not in _NC_CACHE:
        _NC_CACHE[key] = build_program(GROUPS_PER_CORE, repeats=1)
    return _NC_CACHE[key]


def make_in_maps(x: np.ndarray, weights: np.ndarray, blade: np.ndarray):
    xb8, w8, m64b, s_all = prep_inputs(x, weights, blade)
    gpc = GROUPS_PER_CORE
    return [
        {
            "XB": xb8[c * gpc:(c + 1) * gpc],
            "W2": w8,
            "M64B": m64b,
            "SALL": np.ascontiguousarray(s_all[:, c * gpc:(c + 1) * gpc]),
        }
        for c in range(N_CORES)
    ]


def kernel(x: np.ndarray, weights: np.ndarray, blade: np.ndarray) -> np.ndarray:
    in_maps = make_in_maps(x, weights, blade)
    nc = _get_program()
    try:
        res = run_bass_kernel_spmd(nc, in_maps, list(range(N_CORES))).results
    except Exception:
        import time as _time
        _time.sleep(10)
        res = run_bass_kernel_spmd(nc, in_maps, list(range(N_CORES))).results
    outT_all = np.concatenate([res[c]["outT"] for c in range(N_CORES)], axis=0)
    return unprep_output(outT_all)
